# revision 35
# baseline (speedup 1.0000x reference)
"""Multi-head attention TRN2 kernel, head-parallel over 8 NeuronCores.

Problem shape: B=2, S=2048, d_model=512, n_heads=8, head_dim=512
(the projections are Linear(512, 512*8), so each head has dim 512).

Sharding: core h computes head h for both batches (column-parallel
Wq/Wk/Wv, row-parallel Wo).  Each core returns a partial output of
shape [B*S, 512]; the host sums the 8 partials and adds the constant
bv @ Wo + bo (bv passes through attention linearly because softmax
rows sum to 1).

Algebraic folding (exact up to rounding):
  scores = (Q Wq + bq)(K Wk + bk)^T / sqrt(d)
         = Q A K^T / sqrt(d) + row-const + dk^T + const,
  with A = Wq Wk^T and dk = K Wk bq / sqrt(d).  Row-constants cancel
  in softmax, so the device computes u = Q A (one projection), raw-K
  scores, and adds dk as the per-partition bias of the exp activation.
  Likewise out = P (V Wv + bv) Wo / denom = (P V) G / denom + bv Wo
  with G = Wv Wo, so K/V are used raw and only two 512x512 matrices
  (A, G) ship per head.  A, G, dk are computed on the host.

Device layout avoids all on-device transposes:
  - Host passes Q^T, K^T as [512, B*S] bf16 and V as [B*S, 512] bf16.
  - Scores are computed transposed (P^T tiles [Sk, Sq]); exp on ACT
    (no max subtraction: |scores| <= ~2.6 for this problem's scale).
  - PV matmuls with raw V as stationary give (P V)^T directly; softmax
    denominators come from a ones-vector matmul over the same P^T
    tiles; their reciprocal row is transposed into partition layout by
    four tiny K=1 matmuls and applied per-partition after the final G
    projection.  (P V)^T blocks are the stationary operand for G.
"""

import math

import numpy as np
import ml_dtypes

B = 2
S = 2048
D = 512          # d_model == head_dim
H = 8
N_CORES = 8
BS = B * S       # 4096
NT = D // 128    # 4 contraction tiles of 128
SQC = 512        # query-chunk (matmul moving free dim)
NSQ = S // SQC   # 4 chunks per batch
NKT = S // 128   # 16 key tiles per batch
SCALE = 1.0 / math.sqrt(float(D))

_compiled = None


def _body(nc, mybir, pools, aps, pe_only=False, no_out=False, no_den=False):
    f32 = mybir.dt.float32
    bf16 = mybir.dt.bfloat16
    Exp = mybir.ActivationFunctionType.Exp

    class _Skip:
        def __getattr__(self, name):
            return lambda *a, **k: None

    act = _Skip() if pe_only else nc.scalar
    dve = _Skip() if pe_only else nc.vector
    (kvpool, instream, qinp, qpool, ptp, otp, rpool, outp,
     ps_proj, ps_s, ps_o) = pools
    (qt_d, kt_d, vn_d, out_d,
     ah_sb, gh_sb, dk_sb, ones_sb, one1_sb) = aps

    bb = {}
    if pe_only:
        # static SBUF buffers standing in for tiles whose writers
        # (ACT/DVE) are skipped; PE reads garbage, timing-only build
        def _sb(name, shape, dt):
            return nc.alloc_sbuf_tensor(name, list(shape), dt).ap()
        bb["utc"] = _sb("bb_utc", [128, NT, SQC], bf16)
        bb["otc"] = _sb("bb_otc", [128, NT, SQC], bf16)
        bb["rec"] = _sb("bb_rec", [1, SQC], f32)
        for j in range(NKT):
            bb["pt%d" % j] = _sb("bb_pt%d" % j, [128, SQC], bf16)

    def emit_kv(b):
        """K^T tiles (scores stationary) + raw V tiles (PV stationary)
        for batch b.  K loads are column-chunked so the first scores
        matmuls unblock after ~0.5MB instead of the full 4MB."""
        tok0 = S * b
        kin = []
        for t in range(NT):
            ki = instream.tile([128, S], bf16, tag="kin")
            kin.append(ki)
        for cc in range(NSQ):
            for t in range(NT):
                nc.sync.dma_start(
                    kin[t][:, SQC * cc:SQC * (cc + 1)],
                    kt_d[128 * t:128 * (t + 1),
                         tok0 + SQC * cc:tok0 + SQC * (cc + 1)])
        v_b = kvpool.tile([128, NKT, D], bf16, tag="vb")
        for j in range(NKT):
            r0 = tok0 + 128 * j
            nc.sync.dma_start(v_b[:, j, :], vn_d[r0:r0 + 128, :])
        return kin, v_b

    def emit_uproj(b, c):
        """u^T chunk: ut_c[p, m, s] = (Q A)^T[128m+p, q0+s]."""
        q0 = S * b + SQC * c
        qin = []
        for t in range(NT):
            qi = qinp.tile([128, SQC], bf16, tag="qin")
            nc.gpsimd.dma_start(
                qi[:, :], qt_d[128 * t:128 * (t + 1), q0:q0 + SQC])
            qin.append(qi)
        ut_c = bb["utc"] if pe_only else qpool.tile(
            [128, NT, SQC], bf16, tag="utc")
        for m in range(NT):
            ps = ps_proj.tile([128, SQC], f32, tag="psp")
            for t in range(NT):
                nc.tensor.matmul(
                    ps[:, :],
                    ah_sb[:, t, 128 * m:128 * (m + 1)],
                    qin[t][:, :],
                    start=(t == 0), stop=(t == NT - 1))
            dve.tensor_copy(ut_c[:, m, :], ps[:, :])
        return ut_c

    # software-pipelined over (batch, chunk) pairs: the next pair's
    # q-load + u-projection are emitted between the current pair's PV
    # and G passes so the PE never waits at chunk boundaries, and the
    # next batch's K/V loads start one chunk early
    pairs = [(b, c) for b in range(B) for c in range(NSQ)]
    kvs = {0: emit_kv(0)}
    ut_next = emit_uproj(0, 0)

    for i, (b, c) in enumerate(pairs):
        tok0 = S * b
        q0 = tok0 + SQC * c
        kin, v_b = kvs[b]
        ut_c = ut_next

        if True:
            # --- P^T tiles: exp(scale * (K u^T) + dk) ---
            pts = []
            for j in range(NKT):
                ps = ps_s.tile([128, SQC], f32, tag="pss")
                for t in range(NT):
                    nc.tensor.matmul(
                        ps[:, :],
                        kin[t][:, 128 * j:128 * (j + 1)],
                        ut_c[:, t, :],
                        start=(t == 0), stop=(t == NT - 1))
                pt_j = (bb["pt%d" % j] if pe_only
                        else ptp.tile([128, SQC], bf16, tag="ptj"))
                act.activation(pt_j[:, :], ps[:, :], Exp, scale=SCALE,
                               bias=dk_sb[:, b * NKT + j:b * NKT + j + 1])
                pts.append(pt_j)

            # --- softmax denominators: d[sq] = sum_sk P^T[sk, sq].
            # Pairwise tree-sum the P^T tiles on the DVE (elementwise,
            # per-element roundings independent so they average out in
            # d), then contract partitions with ONE ones-matmul --
            # saves 15 of 16 PE matmuls per chunk vs accumulating on
            # the PE. pe_only keeps the all-PE form (DVE skipped). ---
            rcol = None
            if not no_den:
                pd = ps_o.tile([1, SQC], f32, tag="pso")
                if pe_only:
                    for j in range(NKT):
                        nc.tensor.matmul(pd[:, :], ones_sb[:, :],
                                         pts[j][:, :],
                                         start=(j == 0),
                                         stop=(j == NKT - 1))
                else:
                    lvl = pts
                    while len(lvl) > 1:
                        nxt = []
                        for a_, b_ in zip(lvl[0::2], lvl[1::2]):
                            s_ = ptp.tile([128, SQC], bf16, tag="ptsum")
                            dve.tensor_add(s_[:, :], a_[:, :], b_[:, :])
                            nxt.append(s_)
                        lvl = nxt
                    nc.tensor.matmul(pd[:, :], ones_sb[:, :], lvl[0][:, :],
                                     start=True, stop=True)
                rec = (bb["rec"] if pe_only
                       else rpool.tile([1, SQC], f32, tag="rec"))
                dve.reciprocal(rec[:, :], pd[:, :])

            # --- (P V)^T, unnormalized; scaled after the G matmul ---
            ot_c = bb["otc"] if pe_only else otp.tile(
                [128, NT, SQC], bf16, tag="otc")
            for m in range(NT):
                po = ps_o.tile([128, SQC], f32, tag="pso")
                for j in range(NKT):
                    nc.tensor.matmul(
                        po[:, :],
                        v_b[:, j, 128 * m:128 * (m + 1)],
                        pts[j][:, :],
                        start=(j == 0), stop=(j == NKT - 1))
                if not pe_only:
                    dve.tensor_copy(ot_c[:, m, :], po[:, :])
                if m == 0 and not no_den:
                    # transpose rec [1,512] into partition layout
                    # [128,4] via 4 tiny K=1 matmuls (rec slice
                    # stationary, 1x1 ones moving); placed after PV
                    # pass m=0 so the PE never waits on the reciprocal
                    rT_ps = ps_o.tile([128, NSQ], f32, tag="pso")
                    for jq in range(NSQ):
                        nc.tensor.matmul(
                            rT_ps[:, jq:jq + 1],
                            rec[:, 128 * jq:128 * (jq + 1)],
                            one1_sb[:, :], start=True, stop=True)
                    rcol = (None if pe_only
                            else rpool.tile([128, NSQ], f32, tag="rcol"))
                    if not pe_only:
                        dve.tensor_copy(rcol[:, :], rT_ps[:, :])

            # --- pipeline: next batch K/V, next pair's u-projection ---
            if c == NSQ - 2 and b + 1 < B:
                kvs[b + 1] = emit_kv(b + 1)
            if i + 1 < len(pairs):
                nb, nc2 = pairs[i + 1]
                if nc2 == 0 and nb not in kvs:
                    kvs[nb] = emit_kv(nb)
                ut_next = emit_uproj(nb, nc2)

            # --- partial out chunk: ((P V) G) * recip[sq] ---
            for jq in range(NSQ):
                pw = ps_proj.tile([128, D], f32, tag="psp")
                for m in range(NT):
                    nc.tensor.matmul(
                        pw[:, :],
                        ot_c[:, m, 128 * jq:128 * (jq + 1)],
                        gh_sb[:, m, :],
                        start=(m == 0), stop=(m == NT - 1))
                if not pe_only and not no_out:
                    osb = outp.tile([128, D], f32, tag="osb")
                    if no_den:
                        dve.tensor_copy(osb[:, :], pw[:, :])
                    else:
                        dve.tensor_scalar_mul(osb[:, :], pw[:, :],
                                              rcol[:, jq:jq + 1])
                    r0 = q0 + 128 * jq
                    nc.sync.dma_start(out_d[r0:r0 + 128, :], osb[:, :])


def _build(loop_k=1, bench=False, pe_only=False, no_out=False, no_den=False):
    """Build the bass program. loop_k>1 wraps the whole body in a
    For_i hardware loop; bench=True demotes the real output to an
    internal DRAM buffer and emits a tiny dummy ExternalOutput (both
    used only for differential timing in test.py; the graded path uses
    loop_k=1, bench=False)."""
    import contextlib

    import concourse.tile as tile
    from concourse import bacc, mybir

    f32 = mybir.dt.float32
    bf16 = mybir.dt.bfloat16

    nc = bacc.Bacc("TRN2", target_bir_lowering=False, debug=False,
                   num_devices=N_CORES)

    qt_d = nc.dram_tensor("qt", [D, BS], bf16, kind="ExternalInput").ap()
    kt_d = nc.dram_tensor("kt", [D, BS], bf16, kind="ExternalInput").ap()
    vn_d = nc.dram_tensor("vn", [BS, D], bf16, kind="ExternalInput").ap()
    ah_d = nc.dram_tensor("ah", [D, D], bf16, kind="ExternalInput").ap()
    gh_d = nc.dram_tensor("gh", [D, D], bf16, kind="ExternalInput").ap()
    dk_d = nc.dram_tensor("dk", [128, B * NKT], f32,
                          kind="ExternalInput").ap()
    if bench:
        out_d = nc.dram_tensor("outbuf", [BS, D], f32).ap()
        dummy_d = nc.dram_tensor("out", [128, B * NKT], f32,
                                 kind="ExternalOutput").ap()
    else:
        out_d = nc.dram_tensor("out", [BS, D], f32, kind="ExternalOutput").ap()
        dummy_d = None

    with tile.TileContext(nc) as tc:
        with (
            tc.tile_pool(name="weights", bufs=1) as wpool,
            tc.tile_pool(name="kv", bufs=2) as kvpool,
            tc.tile_pool(name="instream", bufs=8) as instream,
            tc.tile_pool(name="qin", bufs=8) as qinp,
            tc.tile_pool(name="q", bufs=3) as qpool,
            tc.tile_pool(name="pt", bufs=24) as ptp,
            tc.tile_pool(name="ot", bufs=3) as otp,
            tc.tile_pool(name="recip", bufs=3) as rpool,
            tc.tile_pool(name="outsb", bufs=8) as outp,
            tc.tile_pool(name="ps_proj", bufs=2, space="PSUM") as ps_proj,
            tc.tile_pool(name="ps_s", bufs=4, space="PSUM") as ps_s,
            tc.tile_pool(name="ps_o", bufs=2, space="PSUM") as ps_o,
        ):
            # --- per-head matrices / biases, resident for the kernel ---
            ah_sb = wpool.tile([128, NT, D], bf16, tag="ah")
            gh_sb = wpool.tile([128, NT, D], bf16, tag="gh")
            for t in range(NT):
                nc.scalar.dma_start(ah_sb[:, t, :], ah_d[128 * t:128 * (t + 1), :])
                nc.scalar.dma_start(gh_sb[:, t, :], gh_d[128 * t:128 * (t + 1), :])
            dk_sb = wpool.tile([128, B * NKT], f32, tag="dk")
            nc.scalar.dma_start(dk_sb[:, :], dk_d[:, :])
            ones_sb = wpool.tile([128, 1], bf16, tag="ones")
            nc.vector.memset(ones_sb[:, :], 1.0)
            one1_sb = wpool.tile([1, 1], f32, tag="one1")
            nc.vector.memset(one1_sb[:, :], 1.0)

            pools = (kvpool, instream, qinp, qpool, ptp, otp, rpool, outp,
                     ps_proj, ps_s, ps_o)
            aps = (qt_d, kt_d, vn_d, out_d,
                   ah_sb, gh_sb, dk_sb, ones_sb, one1_sb)
            loop_cm = (tc.For_i(0, loop_k, 1) if loop_k > 1
                       else contextlib.nullcontext())
            with loop_cm:
                _body(nc, mybir, pools, aps, pe_only=pe_only,
                      no_out=no_out, no_den=no_den)

            if dummy_d is not None:
                nc.sync.dma_start(dummy_d[:, :], dk_sb[:, :])

    nc.compile()
    return nc


def _get_compiled():
    global _compiled
    if _compiled is None:
        _compiled = _build()
    return _compiled


def _make_in_maps(Q, K, V, Wq, bq, Wk, bk, Wv, bv, Wo, bo):
    bf = ml_dtypes.bfloat16
    f32 = np.float32
    Qf = np.ascontiguousarray(Q.reshape(BS, D)).astype(f32)
    Kf = np.ascontiguousarray(K.reshape(BS, D)).astype(f32)
    Vf = np.ascontiguousarray(V.reshape(BS, D)).astype(f32)
    qt = np.ascontiguousarray(Qf.T).astype(bf)
    kt = np.ascontiguousarray(Kf.T).astype(bf)
    vn = Vf.astype(bf)
    in_maps = []
    for h in range(N_CORES):
        sl = slice(D * h, D * (h + 1))
        ah = Wq[:, sl].astype(f32) @ Wk[:, sl].T.astype(f32)
        gh = Wv[:, sl].astype(f32) @ Wo[sl, :].astype(f32)
        dk = (Kf @ (Wk[:, sl].astype(f32) @ bq[sl].astype(f32))) * f32(SCALE)
        in_maps.append({
            "qt": qt, "kt": kt, "vn": vn,
            "ah": np.ascontiguousarray(ah).astype(bf),
            "gh": np.ascontiguousarray(gh).astype(bf),
            "dk": np.ascontiguousarray(dk.reshape(B * NKT, 128).T).astype(f32),
        })
    return in_maps


def kernel(Q, K, V, Wq, bq, Wk, bk, Wv, bv, Wo, bo):
    from concourse.bass_utils import run_bass_kernel_spmd

    # inputs may arrive as jax arrays; keep all host math in numpy
    Q, K, V = np.asarray(Q), np.asarray(K), np.asarray(V)
    Wq, bq = np.asarray(Wq), np.asarray(bq)
    Wk, bk = np.asarray(Wk), np.asarray(bk)
    Wv, bv = np.asarray(Wv), np.asarray(bv)
    Wo, bo = np.asarray(Wo), np.asarray(bo)

    nc = _get_compiled()
    in_maps = _make_in_maps(Q, K, V, Wq, bq, Wk, bk, Wv, bv, Wo, bo)
    res = run_bass_kernel_spmd(nc, in_maps, core_ids=list(range(N_CORES)))
    kernel.last_results = res

    acc = np.zeros((BS, D), np.float64)
    for h in range(N_CORES):
        acc += res.results[h]["out"].astype(np.float64)
    const = bv.astype(np.float64) @ Wo.astype(np.float64) + bo.astype(np.float64)
    return (acc + const).astype(np.float32).reshape(B, S, D)


# revision 36
# speedup vs baseline: 1.0023x; 1.0023x over previous
"""Multi-head attention TRN2 kernel, head-parallel over 8 NeuronCores.

Problem shape: B=2, S=2048, d_model=512, n_heads=8, head_dim=512
(the projections are Linear(512, 512*8), so each head has dim 512).

Sharding: core h computes head h for both batches (column-parallel
Wq/Wk/Wv, row-parallel Wo).  Each core returns a partial output of
shape [B*S, 512]; the host sums the 8 partials and adds the constant
bv @ Wo + bo (bv passes through attention linearly because softmax
rows sum to 1).

Algebraic folding (exact up to rounding):
  scores = (Q Wq + bq)(K Wk + bk)^T / sqrt(d)
         = Q A K^T / sqrt(d) + row-const + dk^T + const,
  with A = Wq Wk^T and dk = K Wk bq / sqrt(d).  Row-constants cancel
  in softmax, so the device computes u = Q A (one projection), raw-K
  scores, and adds dk as the per-partition bias of the exp activation.
  Likewise out = P (V Wv + bv) Wo / denom = (P V) G / denom + bv Wo
  with G = Wv Wo, so K/V are used raw and only two 512x512 matrices
  (A, G) ship per head.  A, G, dk are computed on the host.

Device layout avoids all on-device transposes:
  - Host passes Q^T, K^T as [512, B*S] bf16 and V as [B*S, 512] bf16.
  - Scores are computed transposed (P^T tiles [Sk, Sq]); exp on ACT
    (no max subtraction: |scores| <= ~2.6 for this problem's scale).
  - PV matmuls with raw V as stationary give (P V)^T directly; softmax
    denominators come from a ones-vector matmul over the same P^T
    tiles; their reciprocal row is transposed into partition layout by
    four tiny K=1 matmuls and applied per-partition after the final G
    projection.  (P V)^T blocks are the stationary operand for G.
"""

import math

import numpy as np
import ml_dtypes

B = 2
S = 2048
D = 512          # d_model == head_dim
H = 8
N_CORES = 8
BS = B * S       # 4096
NT = D // 128    # 4 contraction tiles of 128
SQC = 512        # query-chunk (matmul moving free dim)
NSQ = S // SQC   # 4 chunks per batch
NKT = S // 128   # 16 key tiles per batch
SCALE = 1.0 / math.sqrt(float(D))

_compiled = None


def _body(nc, mybir, pools, aps, pe_only=False, no_out=False, no_den=False):
    f32 = mybir.dt.float32
    bf16 = mybir.dt.bfloat16
    Exp = mybir.ActivationFunctionType.Exp

    class _Skip:
        def __getattr__(self, name):
            return lambda *a, **k: None

    act = _Skip() if pe_only else nc.scalar
    dve = _Skip() if pe_only else nc.vector
    (kvpool, instream, qinp, qpool, ptp, otp, rpool, outp,
     ps_proj, ps_s, ps_o) = pools
    (qt_d, kt_d, vn_d, out_d,
     ah_sb, gh_sb, dk_sb, ones_sb, one1_sb) = aps

    bb = {}
    if pe_only:
        # static SBUF buffers standing in for tiles whose writers
        # (ACT/DVE) are skipped; PE reads garbage, timing-only build
        def _sb(name, shape, dt):
            return nc.alloc_sbuf_tensor(name, list(shape), dt).ap()
        bb["utc"] = _sb("bb_utc", [128, NT, SQC], bf16)
        bb["otc"] = _sb("bb_otc", [128, NT, SQC], bf16)
        bb["rec"] = _sb("bb_rec", [1, SQC], f32)
        for j in range(NKT):
            bb["pt%d" % j] = _sb("bb_pt%d" % j, [128, SQC], bf16)

    def emit_kv(b):
        """K^T tiles (scores stationary) + raw V tiles (PV stationary)
        for batch b.  K loads are column-chunked so the first scores
        matmuls unblock after ~0.5MB instead of the full 4MB."""
        tok0 = S * b
        kin = []
        for t in range(NT):
            ki = instream.tile([128, S], bf16, tag="kin")
            kin.append(ki)
        for cc in range(NSQ):
            for t in range(NT):
                nc.sync.dma_start(
                    kin[t][:, SQC * cc:SQC * (cc + 1)],
                    kt_d[128 * t:128 * (t + 1),
                         tok0 + SQC * cc:tok0 + SQC * (cc + 1)])
        v_b = kvpool.tile([128, NKT, D], bf16, tag="vb")
        for j in range(NKT):
            r0 = tok0 + 128 * j
            nc.sync.dma_start(v_b[:, j, :], vn_d[r0:r0 + 128, :])
        return kin, v_b

    def emit_uproj(b, c):
        """u^T chunk: ut_c[p, m, s] = (Q A)^T[128m+p, q0+s]."""
        q0 = S * b + SQC * c
        qin = []
        for t in range(NT):
            qi = qinp.tile([128, SQC], bf16, tag="qin")
            nc.gpsimd.dma_start(
                qi[:, :], qt_d[128 * t:128 * (t + 1), q0:q0 + SQC])
            qin.append(qi)
        ut_c = bb["utc"] if pe_only else qpool.tile(
            [128, NT, SQC], bf16, tag="utc")
        for m in range(NT):
            ps = ps_proj.tile([128, SQC], f32, tag="psp")
            for t in range(NT):
                nc.tensor.matmul(
                    ps[:, :],
                    ah_sb[:, t, 128 * m:128 * (m + 1)],
                    qin[t][:, :],
                    start=(t == 0), stop=(t == NT - 1))
            dve.tensor_copy(ut_c[:, m, :], ps[:, :])
        return ut_c

    # software-pipelined over (batch, chunk) pairs: the next pair's
    # q-load + u-projection are emitted between the current pair's PV
    # and G passes so the PE never waits at chunk boundaries, and the
    # next batch's K/V loads start one chunk early
    pairs = [(b, c) for b in range(B) for c in range(NSQ)]
    kvs = {0: emit_kv(0)}
    ut_next = emit_uproj(0, 0)

    for i, (b, c) in enumerate(pairs):
        tok0 = S * b
        q0 = tok0 + SQC * c
        kin, v_b = kvs[b]
        ut_c = ut_next

        if True:
            # --- P^T tiles: exp(scale * (K u^T) + dk) ---
            pts = []
            for j in range(NKT):
                ps = ps_s.tile([128, SQC], f32, tag="pss")
                for t in range(NT):
                    nc.tensor.matmul(
                        ps[:, :],
                        kin[t][:, 128 * j:128 * (j + 1)],
                        ut_c[:, t, :],
                        start=(t == 0), stop=(t == NT - 1))
                pt_j = (bb["pt%d" % j] if pe_only
                        else ptp.tile([128, SQC], bf16, tag="ptj"))
                act.activation(pt_j[:, :], ps[:, :], Exp, scale=SCALE,
                               bias=dk_sb[:, b * NKT + j:b * NKT + j + 1])
                pts.append(pt_j)

            # --- softmax denominators: d[sq] = sum_sk P^T[sk, sq] ---
            rcol = None
            if not no_den:
                pd = ps_o.tile([1, SQC], f32, tag="pso")
                for j in range(NKT):
                    nc.tensor.matmul(pd[:, :], ones_sb[:, :], pts[j][:, :],
                                     start=(j == 0), stop=(j == NKT - 1))
                rec = (bb["rec"] if pe_only
                       else rpool.tile([1, SQC], f32, tag="rec"))
                dve.reciprocal(rec[:, :], pd[:, :])

            # --- (P V)^T, unnormalized; scaled after the G matmul ---
            ot_c = bb["otc"] if pe_only else otp.tile(
                [128, NT, SQC], bf16, tag="otc")
            for m in range(NT):
                po = ps_o.tile([128, SQC], f32, tag="pso")
                for j in range(NKT):
                    nc.tensor.matmul(
                        po[:, :],
                        v_b[:, j, 128 * m:128 * (m + 1)],
                        pts[j][:, :],
                        start=(j == 0), stop=(j == NKT - 1))
                if not pe_only:
                    dve.tensor_copy(ot_c[:, m, :], po[:, :])
                if m == 0 and not no_den:
                    # transpose rec [1,512] into partition layout
                    # [128,4] via 4 tiny K=1 matmuls (rec slice
                    # stationary, 1x1 ones moving); placed after PV
                    # pass m=0 so the PE never waits on the reciprocal
                    rT_ps = ps_o.tile([128, NSQ], f32, tag="pso")
                    for jq in range(NSQ):
                        nc.tensor.matmul(
                            rT_ps[:, jq:jq + 1],
                            rec[:, 128 * jq:128 * (jq + 1)],
                            one1_sb[:, :], start=True, stop=True)
                    rcol = (None if pe_only
                            else rpool.tile([128, NSQ], f32, tag="rcol"))
                    if not pe_only:
                        dve.tensor_copy(rcol[:, :], rT_ps[:, :])

            # --- pipeline: next batch K/V, next pair's u-projection ---
            if c == NSQ - 2 and b + 1 < B:
                kvs[b + 1] = emit_kv(b + 1)
            if i + 1 < len(pairs):
                nb, nc2 = pairs[i + 1]
                if nc2 == 0 and nb not in kvs:
                    kvs[nb] = emit_kv(nb)
                ut_next = emit_uproj(nb, nc2)

            # --- partial out chunk: ((P V) G) * recip[sq] ---
            for jq in range(NSQ):
                pw = ps_proj.tile([128, D], f32, tag="psp")
                for m in range(NT):
                    nc.tensor.matmul(
                        pw[:, :],
                        ot_c[:, m, 128 * jq:128 * (jq + 1)],
                        gh_sb[:, m, :],
                        start=(m == 0), stop=(m == NT - 1))
                if not pe_only and not no_out:
                    osb = outp.tile([128, D], f32, tag="osb")
                    if no_den:
                        dve.tensor_copy(osb[:, :], pw[:, :])
                    else:
                        dve.tensor_scalar_mul(osb[:, :], pw[:, :],
                                              rcol[:, jq:jq + 1])
                    r0 = q0 + 128 * jq
                    nc.sync.dma_start(out_d[r0:r0 + 128, :], osb[:, :])


def _build(loop_k=1, bench=False, pe_only=False, no_out=False, no_den=False):
    """Build the bass program. loop_k>1 wraps the whole body in a
    For_i hardware loop; bench=True demotes the real output to an
    internal DRAM buffer and emits a tiny dummy ExternalOutput (both
    used only for differential timing in test.py; the graded path uses
    loop_k=1, bench=False)."""
    import contextlib

    import concourse.tile as tile
    from concourse import bacc, mybir

    f32 = mybir.dt.float32
    bf16 = mybir.dt.bfloat16

    nc = bacc.Bacc("TRN2", target_bir_lowering=False, debug=False,
                   num_devices=N_CORES)

    qt_d = nc.dram_tensor("qt", [D, BS], bf16, kind="ExternalInput").ap()
    kt_d = nc.dram_tensor("kt", [D, BS], bf16, kind="ExternalInput").ap()
    vn_d = nc.dram_tensor("vn", [BS, D], bf16, kind="ExternalInput").ap()
    ah_d = nc.dram_tensor("ah", [D, D], bf16, kind="ExternalInput").ap()
    gh_d = nc.dram_tensor("gh", [D, D], bf16, kind="ExternalInput").ap()
    dk_d = nc.dram_tensor("dk", [128, B * NKT], f32,
                          kind="ExternalInput").ap()
    if bench:
        out_d = nc.dram_tensor("outbuf", [BS, D], f32).ap()
        dummy_d = nc.dram_tensor("out", [128, B * NKT], f32,
                                 kind="ExternalOutput").ap()
    else:
        out_d = nc.dram_tensor("out", [BS, D], f32, kind="ExternalOutput").ap()
        dummy_d = None

    with tile.TileContext(nc) as tc:
        with (
            tc.tile_pool(name="weights", bufs=1) as wpool,
            tc.tile_pool(name="kv", bufs=2) as kvpool,
            tc.tile_pool(name="instream", bufs=8) as instream,
            tc.tile_pool(name="qin", bufs=8) as qinp,
            tc.tile_pool(name="q", bufs=3) as qpool,
            tc.tile_pool(name="pt", bufs=24) as ptp,
            tc.tile_pool(name="ot", bufs=3) as otp,
            tc.tile_pool(name="recip", bufs=3) as rpool,
            tc.tile_pool(name="outsb", bufs=8) as outp,
            tc.tile_pool(name="ps_proj", bufs=2, space="PSUM") as ps_proj,
            tc.tile_pool(name="ps_s", bufs=4, space="PSUM") as ps_s,
            tc.tile_pool(name="ps_o", bufs=2, space="PSUM") as ps_o,
        ):
            # --- per-head matrices / biases, resident for the kernel ---
            ah_sb = wpool.tile([128, NT, D], bf16, tag="ah")
            gh_sb = wpool.tile([128, NT, D], bf16, tag="gh")
            for t in range(NT):
                nc.scalar.dma_start(ah_sb[:, t, :], ah_d[128 * t:128 * (t + 1), :])
                nc.scalar.dma_start(gh_sb[:, t, :], gh_d[128 * t:128 * (t + 1), :])
            dk_sb = wpool.tile([128, B * NKT], f32, tag="dk")
            nc.scalar.dma_start(dk_sb[:, :], dk_d[:, :])
            ones_sb = wpool.tile([128, 1], bf16, tag="ones")
            nc.vector.memset(ones_sb[:, :], 1.0)
            one1_sb = wpool.tile([1, 1], f32, tag="one1")
            nc.vector.memset(one1_sb[:, :], 1.0)

            pools = (kvpool, instream, qinp, qpool, ptp, otp, rpool, outp,
                     ps_proj, ps_s, ps_o)
            aps = (qt_d, kt_d, vn_d, out_d,
                   ah_sb, gh_sb, dk_sb, ones_sb, one1_sb)
            loop_cm = (tc.For_i(0, loop_k, 1) if loop_k > 1
                       else contextlib.nullcontext())
            with loop_cm:
                _body(nc, mybir, pools, aps, pe_only=pe_only,
                      no_out=no_out, no_den=no_den)

            if dummy_d is not None:
                nc.sync.dma_start(dummy_d[:, :], dk_sb[:, :])

    nc.compile()
    return nc


def _get_compiled():
    global _compiled
    if _compiled is None:
        _compiled = _build()
    return _compiled


def _make_in_maps(Q, K, V, Wq, bq, Wk, bk, Wv, bv, Wo, bo):
    bf = ml_dtypes.bfloat16
    f32 = np.float32
    Qf = np.ascontiguousarray(Q.reshape(BS, D)).astype(f32)
    Kf = np.ascontiguousarray(K.reshape(BS, D)).astype(f32)
    Vf = np.ascontiguousarray(V.reshape(BS, D)).astype(f32)
    qt = np.ascontiguousarray(Qf.T).astype(bf)
    kt = np.ascontiguousarray(Kf.T).astype(bf)
    vn = Vf.astype(bf)
    in_maps = []
    for h in range(N_CORES):
        sl = slice(D * h, D * (h + 1))
        ah = Wq[:, sl].astype(f32) @ Wk[:, sl].T.astype(f32)
        gh = Wv[:, sl].astype(f32) @ Wo[sl, :].astype(f32)
        dk = (Kf @ (Wk[:, sl].astype(f32) @ bq[sl].astype(f32))) * f32(SCALE)
        in_maps.append({
            "qt": qt, "kt": kt, "vn": vn,
            "ah": np.ascontiguousarray(ah).astype(bf),
            "gh": np.ascontiguousarray(gh).astype(bf),
            "dk": np.ascontiguousarray(dk.reshape(B * NKT, 128).T).astype(f32),
        })
    return in_maps


def kernel(Q, K, V, Wq, bq, Wk, bk, Wv, bv, Wo, bo):
    from concourse.bass_utils import run_bass_kernel_spmd

    # inputs may arrive as jax arrays; keep all host math in numpy
    Q, K, V = np.asarray(Q), np.asarray(K), np.asarray(V)
    Wq, bq = np.asarray(Wq), np.asarray(bq)
    Wk, bk = np.asarray(Wk), np.asarray(bk)
    Wv, bv = np.asarray(Wv), np.asarray(bv)
    Wo, bo = np.asarray(Wo), np.asarray(bo)

    nc = _get_compiled()
    in_maps = _make_in_maps(Q, K, V, Wq, bq, Wk, bk, Wv, bv, Wo, bo)
    res = run_bass_kernel_spmd(nc, in_maps, core_ids=list(range(N_CORES)))
    kernel.last_results = res

    acc = np.zeros((BS, D), np.float64)
    for h in range(N_CORES):
        acc += res.results[h]["out"].astype(np.float64)
    const = bv.astype(np.float64) @ Wo.astype(np.float64) + bo.astype(np.float64)
    return (acc + const).astype(np.float32).reshape(B, S, D)
